# revision 10
# baseline (speedup 1.0000x reference)
"""Self-contained Trainium2 Bass kernel for GQA attention (B=2, T=2048, D=4096,
32 q heads / 8 kv heads, HD=128, RoPE, no causal mask, start_pos=0).

Sharding: 8 cores = 2 (batch) x 4 (head groups). Each core computes 8 q heads /
2 kv heads for one batch and a partial o-projection; the host sums the 4
partials per batch.

All matmul operands are bf16 (PSUM accumulation stays f32), which halves DMA
traffic and lets every weight stay SBUF-resident (loaded exactly once).

Schedule (single core), tuned so the tensor engine never waits on the
exp-activation engine:
  A      : k/v projections + RoPE for all 4 t-chunks (x streamed, w resident)
  slot0  : q projection chunk 0
  slot1-3: attention(chunk c-1) with q-projection(chunk c) matmuls interleaved
  slot4  : attention(chunk 3) with o-projection(chunk 0) interleaved
  C      : o-projection chunks 1-3
Attention per (head, s-block): scores matmul -> exp on ACT engine (bf16 out) ->
ctx matmul (lagged one iteration so exp latency is hidden). The softmax
denominator never touches the tensor engine: a pairwise tree-add of exp tiles
on DVE, a GpSimd partition_all_reduce across s-partitions, DVE reciprocal, and
a DVE broadcast-multiply normalize ctx.

RoPE: wq/wk rows are permuted on the host so each head's (re, im) pairs sit 16
partitions apart within a 32-partition quadrant; stream_shuffle swaps them and
two multiplies + add with host-built cos/sin tables apply the rotation.
"""

import sys
import math

for _p in ("/opt/trn_rl_repo", "/root/.axon_site"):
    if _p not in sys.path:
        sys.path.insert(0, _p)

import numpy as np

T = 2048
D = 4096
N_HEADS = 32
N_KV = 8
HD = 128
N_CORES = 8
GQ = N_HEADS // 4   # q heads per core = 8
GKV = N_KV // 4     # kv heads per core = 2
N_REP = GQ // GKV   # 4
TCH = 512           # t-chunk
NCH = T // TCH      # 4
KT = D // 128       # 32 contraction tiles
NSB = T // 128      # 16 s-blocks
QD = GQ * HD        # 1024
KD = GKV * HD       # 256
SCALE = 1.0 / math.sqrt(HD)


def _build_program():
    import concourse.bass as bass
    import concourse.tile as tile
    from concourse import bacc, mybir
    from concourse.bass_isa import ReduceOp

    f32 = mybir.dt.float32
    bf16 = mybir.dt.bfloat16
    Exp = mybir.ActivationFunctionType.Exp

    nc = bacc.Bacc("TRN2", target_bir_lowering=False, debug=False,
                   num_devices=N_CORES)

    xT = nc.dram_tensor("xT", [D, T], bf16, kind="ExternalInput")
    wqT = nc.dram_tensor("wqT", [D, QD], bf16, kind="ExternalInput")
    wkvT = nc.dram_tensor("wkvT", [D, 2 * KD], bf16, kind="ExternalInput")
    woT = nc.dram_tensor("woT", [QD, D], bf16, kind="ExternalInput")
    C2 = nc.dram_tensor("C2", [128, T], bf16, kind="ExternalInput")
    S2m = nc.dram_tensor("S2m", [128, T], bf16, kind="ExternalInput")
    yT = nc.dram_tensor("yT", [D, T], f32, kind="ExternalOutput")

    SWAP = [(i + 16) % 32 for i in range(32)]  # swap 16-halves in each quadrant

    with tile.TileContext(nc) as tc, nc.allow_low_precision("bf16 kernel"):
        with tc.tile_pool(name="persist", bufs=1) as persist:
            k_sb = [persist.tile([128, T], bf16, name=f"k{m}", tag=f"k{m}")
                    for m in range(GKV)]
            v_sb = [persist.tile([128, KD], bf16, name=f"v{tb}", tag=f"v{tb}")
                    for tb in range(NSB)]
            ctx_sb = [[persist.tile([128, TCH], bf16, name=f"ctx{c}_{h}",
                                    tag=f"ctx{c}_{h}") for h in range(GQ)]
                      for c in range(NCH)]
            q_sb = [[persist.tile([128, TCH], bf16, name=f"q{p}_{h}",
                                  tag=f"q{p}_{h}") for h in range(GQ)]
                    for p in range(2)]

            def make_rope(ropep, c2_sb, s2m_sb):
                def rope_evac(ps, dst_ap, c):
                    c0, c1 = c * TCH, (c + 1) * TCH
                    t1 = ropep.tile([128, TCH], bf16, name="rt1", tag="rt1")
                    nc.vector.tensor_mul(t1[:], ps[:], c2_sb[:, c0:c1])
                    sh = ropep.tile([128, TCH], f32, name="rsh", tag="rsh")
                    nc.vector.stream_shuffle(sh[:], ps[:], SWAP)
                    t2 = ropep.tile([128, TCH], bf16, name="rt2", tag="rt2")
                    nc.vector.tensor_mul(t2[:], sh[:], s2m_sb[:, c0:c1])
                    nc.vector.tensor_add(dst_ap, t1[:], t2[:])
                return rope_evac

            def make_att(scp, ctxp, exps, gtmp, rbp):
                """Emit one attention chunk with a filler callable interleaved.

                ctx matmuls lag TWO iterations behind their exp so the ACT
                chain latency never stalls the tensor engine (keeps the PE
                p-state ramped). Softmax denominator: exp pair-adds on GpSimd,
                the small remaining tree + reciprocal + normalize on DVE.
                """
                state = {}
                pend = []

                def emit_ctx(ca, h, sb, ex):
                    st = state[h]
                    kv = h // N_REP
                    nc.tensor.matmul(st["ctx"][:], v_sb[sb][:, kv * 128:(kv + 1) * 128],
                                     ex[:], start=(sb == 0), stop=(sb == NSB - 1))
                    if sb == NSB - 1:
                        den_bc = rbp.tile([128, TCH], f32, name="denbc", tag="denbc")
                        nc.gpsimd.partition_all_reduce(den_bc[:], st["acc"][:],
                                                       128, ReduceOp.add)
                        rb = rbp.tile([128, TCH], f32, name="rb", tag="rb")
                        nc.vector.reciprocal(rb[:], den_bc[:])
                        nc.vector.tensor_mul(ctx_sb[ca][h][:], st["ctx"][:], rb[:])
                        del state[h]

                def att_chunk(ca, filler):
                    for i in range(GQ * NSB):
                        h, sb = divmod(i, NSB)
                        kv = h // N_REP
                        sc = scp.tile([128, TCH], f32, name="sc", tag="sc")
                        nc.tensor.matmul(sc[:], k_sb[kv][:, sb * 128:(sb + 1) * 128],
                                         q_sb[ca % 2][h][:], start=True, stop=True)
                        ex = exps.tile([128, TCH], bf16, name="ex", tag="ex")
                        nc.scalar.activation(ex[:], sc[:], Exp, scale=SCALE)
                        if sb == 0:
                            state[h] = {
                                "ctx": ctxp.tile([128, TCH], f32, name="ctxps", tag="ctxps"),
                                "exs": [], "pairs": [], "quads": [],
                            }
                        st = state[h]
                        st["exs"].append(ex)
                        filler(i)
                        while len(pend) >= 2:
                            emit_ctx(*pend.pop(0))
                        pend.append((ca, h, sb, ex))
                        if sb % 2 == 1:
                            # pair-add the last two exp tiles on the idle GpSimd
                            e0, e1 = st["exs"][-2:]
                            pr = gtmp.tile([128, TCH], bf16, name="pr", tag="pr", bufs=2)
                            nc.gpsimd.tensor_add(pr[:], e0[:], e1[:])
                            st["pairs"].append(pr)
                        if sb % 4 == 3:
                            p0, p1 = st["pairs"][-2:]
                            qd = gtmp.tile([128, TCH], bf16, name="qd", tag="qd", bufs=4)
                            nc.vector.tensor_add(qd[:], p0[:], p1[:])
                            st["quads"].append(qd)
                        if sb == NSB - 1:
                            q0, q1, q2, q3 = st["quads"]
                            h0 = gtmp.tile([128, TCH], bf16, name="h0", tag="h0", bufs=1)
                            nc.vector.tensor_add(h0[:], q0[:], q1[:])
                            h1 = gtmp.tile([128, TCH], bf16, name="h1", tag="h1", bufs=1)
                            nc.vector.tensor_add(h1[:], q2[:], q3[:])
                            acc = gtmp.tile([128, TCH], bf16, name="acc", tag="acc")
                            nc.vector.tensor_add(acc[:], h0[:], h1[:])
                            st["acc"] = acc
                    # flush final pending ctx matmuls
                    while pend:
                        emit_ctx(*pend.pop(0))
                return att_chunk

            # ================= region 1: projections + attention ==========
            with tc.tile_pool(name="wsb", bufs=1) as wsb:
                wq_t = [wsb.tile([128, QD], bf16, name=f"wq{k}", tag=f"wq{k}")
                        for k in range(KT)]
                wkv_t = [wsb.tile([128, 2 * KD], bf16, name=f"wkv{k}", tag=f"wkv{k}")
                         for k in range(KT)]
                c2_sb = wsb.tile([128, T], bf16, tag="c2")
                nc.sync.dma_start(c2_sb[:], C2[:])
                s2m_sb = wsb.tile([128, T], bf16, tag="s2m")
                nc.sync.dma_start(s2m_sb[:], S2m[:])

                # ---------------- Stage A: k/v projections ---------------
                with tc.tile_pool(name="xa", bufs=8) as xap, \
                     tc.tile_pool(name="ropea", bufs=2) as ropea, \
                     tc.tile_pool(name="kvk", bufs=4, space="PSUM") as kvk, \
                     tc.tile_pool(name="kvv", bufs=4, space="PSUM") as kvv:
                    rope_a = make_rope(ropea, c2_sb, s2m_sb)
                    for k in range(3):
                        nc.sync.dma_start(wkv_t[k][:],
                                          wkvT[k * 128:(k + 1) * 128, :])
                    for c in range(NCH):
                        c0, c1 = c * TCH, (c + 1) * TCH
                        kps = [kvk.tile([128, TCH], f32, name=f"kps{m}", tag="kps")
                               for m in range(GKV)]
                        vps = [kvv.tile([128, KD], f32, name=f"vps{tb}", tag="vps")
                               for tb in range(TCH // 128)]
                        for k in range(KT):
                            if c == 0 and k + 3 < KT:
                                # stream weight loads behind the first chunk's
                                # compute instead of blocking at program start
                                nc.sync.dma_start(wkv_t[k + 3][:],
                                                  wkvT[(k + 3) * 128:(k + 4) * 128, :])
                            elif c == 1:
                                nc.sync.dma_start(wq_t[k][:],
                                                  wqT[k * 128:(k + 1) * 128, :])
                            xt = xap.tile([128, TCH], bf16, name="xa", tag="xa")
                            nc.sync.dma_start(xt[:], xT[k * 128:(k + 1) * 128, c0:c1])
                            for m in range(GKV):
                                nc.tensor.matmul(kps[m][:],
                                                 wkv_t[k][:, m * 128:(m + 1) * 128],
                                                 xt[:], start=(k == 0), stop=(k == KT - 1))
                            for tb in range(TCH // 128):
                                nc.tensor.matmul(vps[tb][:],
                                                 xt[:, tb * 128:(tb + 1) * 128],
                                                 wkv_t[k][:, KD:], start=(k == 0),
                                                 stop=(k == KT - 1))
                        for m in range(GKV):
                            rope_a(kps[m], k_sb[m][:, c0:c1], c)
                        for tb in range(TCH // 128):
                            nc.scalar.copy(v_sb[c * (TCH // 128) + tb][:], vps[tb][:])

                # ------------- Stage B: q-proj + attention ----------------
                with tc.tile_pool(name="xb", bufs=7) as xbp, \
                     tc.tile_pool(name="ropeb", bufs=2) as ropeb, \
                     tc.tile_pool(name="exps", bufs=5) as exps, \
                     tc.tile_pool(name="gtmp", bufs=2) as gtmp, \
                     tc.tile_pool(name="rbp", bufs=2) as rbp, \
                     tc.tile_pool(name="qps", bufs=4, space="PSUM") as qpsp, \
                     tc.tile_pool(name="scp", bufs=2, space="PSUM") as scp, \
                     tc.tile_pool(name="ctxp", bufs=2, space="PSUM") as ctxp:
                    rope_b = make_rope(ropeb, c2_sb, s2m_sb)
                    att_chunk = make_att(scp, ctxp, exps, gtmp, rbp)
                    qst = [None] * 4

                    def qproj_kstep(c, p, k):
                        c0, c1 = c * TCH, (c + 1) * TCH
                        if k == 0:
                            for j in range(4):
                                qst[j] = qpsp.tile([128, TCH], f32,
                                                   name=f"qps{j}", tag="qps")
                        xt = xbp.tile([128, TCH], bf16, name="xb", tag="xb")
                        nc.sync.dma_start(xt[:], xT[k * 128:(k + 1) * 128, c0:c1])
                        for j in range(4):
                            h = p * 4 + j
                            nc.tensor.matmul(qst[j][:],
                                             wq_t[k][:, h * 128:(h + 1) * 128],
                                             xt[:], start=(k == 0), stop=(k == KT - 1))
                        if k == KT - 1:
                            for j in range(4):
                                rope_b(qst[j], q_sb[c % 2][p * 4 + j][:], c)

                    # slot 0: q-projection chunk 0, nothing to interleave
                    for p in range(2):
                        for k in range(KT):
                            qproj_kstep(0, p, k)

                    # slots 1..3: attention(c-1) + q-projection(c)
                    for c in range(1, NCH):
                        def filler(i, c=c):
                            if i % 2 == 0:
                                qproj_kstep(c, i // 64, (i % 64) // 2)
                        att_chunk(c - 1, filler)

            # ============ region 2: last attention + o-projection =========
            with tc.tile_pool(name="wo", bufs=1) as wop, \
                 tc.tile_pool(name="exps2", bufs=6) as exps2, \
                 tc.tile_pool(name="gtmp2", bufs=2) as gtmp2, \
                 tc.tile_pool(name="rbp2", bufs=2) as rbp2, \
                 tc.tile_pool(name="out", bufs=4) as outp, \
                 tc.tile_pool(name="scp2", bufs=2, space="PSUM") as scp2, \
                 tc.tile_pool(name="ctxp2", bufs=2, space="PSUM") as ctxp2, \
                 tc.tile_pool(name="yps", bufs=4, space="PSUM") as yps:
                wo_t = [wop.tile([128, D], bf16, name=f"wo{hk}", tag=f"wo{hk}")
                        for hk in range(GQ)]
                for hk in range(GQ):
                    nc.sync.dma_start(wo_t[hk][:], woT[hk * 128:(hk + 1) * 128, :])

                ydone = [None]

                def oproj_mblock(m, c):
                    c0, c1 = c * TCH, (c + 1) * TCH
                    y_ps = yps.tile([128, TCH], f32, name="yps", tag="yps")
                    for hk in range(GQ):
                        nc.tensor.matmul(y_ps[:], wo_t[hk][:, m * 128:(m + 1) * 128],
                                         ctx_sb[c][hk][:], start=(hk == 0),
                                         stop=(hk == GQ - 1))
                    ot = outp.tile([128, TCH], f32, name="ot", tag="ot")
                    nc.scalar.copy(ot[:], y_ps[:])
                    nc.sync.dma_start(yT[m * 128:(m + 1) * 128, c0:c1], ot[:])

                ostate = [None]

                def oproj_part(i):
                    # two o-proj matmuls per attention iteration, chunk 0
                    m, sub = divmod(i, 4)
                    if sub == 0:
                        ostate[0] = yps.tile([128, TCH], f32, name="yps", tag="yps")
                    y_ps = ostate[0]
                    for hk in (2 * sub, 2 * sub + 1):
                        nc.tensor.matmul(y_ps[:], wo_t[hk][:, m * 128:(m + 1) * 128],
                                         ctx_sb[0][hk][:], start=(hk == 0),
                                         stop=(hk == GQ - 1))
                    if sub == 3:
                        ot = outp.tile([128, TCH], f32, name="ot", tag="ot")
                        nc.vector.tensor_copy(ot[:], y_ps[:])
                        nc.sync.dma_start(yT[m * 128:(m + 1) * 128, 0:TCH], ot[:])

                att_chunk2 = make_att(scp2, ctxp2, exps2, gtmp2, rbp2)
                att_chunk2(NCH - 1, oproj_part)

                # Stage C: o-projection chunks 1..3
                for c in range(1, NCH):
                    for m in range(KT):
                        oproj_mblock(m, c)

    nc.compile()
    return nc


_PROGRAM = None


def _get_program():
    global _PROGRAM
    if _PROGRAM is None:
        _PROGRAM = _build_program()
    return _PROGRAM


def _rope_perm():
    """Within-head row permutation: row 32*q + i  <-  component 2*(16q+i%16)+ (i>=16)."""
    perm = np.empty(HD, dtype=np.int64)
    for q in range(4):
        for i in range(32):
            j = 16 * q + (i % 16)
            perm[32 * q + i] = 2 * j + (1 if i >= 16 else 0)
    return perm


def _host_prep(x, wq, wk, wv, wo, cos, sin):
    """Build the per-core input maps."""
    import ml_dtypes
    bf16 = ml_dtypes.bfloat16
    perm = _rope_perm()
    f32 = np.float32

    cosT = np.ascontiguousarray(cos.T.astype(f32))   # [64, T]
    sinT = np.ascontiguousarray(sin.T.astype(f32))
    C2 = np.empty((128, T), f32)
    S2m = np.empty((128, T), f32)
    for q in range(4):
        for i in range(32):
            j = 16 * q + (i % 16)
            C2[32 * q + i] = cosT[j]
            S2m[32 * q + i] = sinT[j] if i >= 16 else -sinT[j]
    C2 = C2.astype(bf16)
    S2m = S2m.astype(bf16)

    in_maps = []
    for core in range(N_CORES):
        b, g = divmod(core, 4)
        qrows = np.concatenate([(8 * g + j) * HD + perm for j in range(GQ)])
        krows = np.concatenate([(2 * g + m) * HD + perm for m in range(GKV)])
        vrows = np.arange(2 * g * HD, (2 * g + 2) * HD)
        ocols = np.arange(8 * g * HD, (8 * g + 8) * HD)
        in_maps.append({
            "xT": np.ascontiguousarray(x[b].T).astype(bf16),
            "wqT": np.ascontiguousarray(wq[qrows].T).astype(bf16),
            "wkvT": np.ascontiguousarray(
                np.concatenate([wk[krows], wv[vrows]], axis=0).T).astype(bf16),
            "woT": np.ascontiguousarray(wo[:, ocols].T).astype(bf16),
            "C2": C2, "S2m": S2m,
        })
    return in_maps


def kernel(x, wq, wk, wv, wo, cache_k, cache_v, cos, sin, mask, start_pos):
    x = np.asarray(x)
    wq, wk, wv, wo = (np.asarray(a) for a in (wq, wk, wv, wo))
    cos, sin = np.asarray(cos), np.asarray(sin)
    assert int(start_pos) == 0, "kernel hardcodes start_pos == 0"
    assert x.shape == (2, T, D)

    from concourse.bass_utils import run_bass_kernel_spmd

    nc = _get_program()
    in_maps = _host_prep(x, wq, wk, wv, wo, cos, sin)
    res = run_bass_kernel_spmd(nc, in_maps, list(range(N_CORES)))

    y = np.empty((2, T, D), np.float32)
    for b in range(2):
        acc = res.results[4 * b]["yT"].copy()
        for g in range(1, 4):
            acc += res.results[4 * b + g]["yT"]
        y[b] = acc.T
    return y


# revision 12
# speedup vs baseline: 1.3044x; 1.3044x over previous
"""Self-contained Trainium2 Bass kernel for GQA attention (B=2, T=2048, D=4096,
32 q heads / 8 kv heads, HD=128, RoPE, no causal mask, start_pos=0).

Sharding: 8 cores = 2 (batch) x 4 (head groups). Each core computes 8 q heads /
2 kv heads for one batch and a partial o-projection; the host sums the 4
partials per batch.

All matmul operands are bf16 (PSUM accumulation stays f32), which halves DMA
traffic and lets every weight stay SBUF-resident (loaded exactly once).

Schedule (single core), tuned so the tensor engine never waits on the
exp-activation engine:
  A      : k/v projections + RoPE for all 4 t-chunks (x streamed, w resident)
  slot0  : q projection chunk 0
  slot1-3: attention(chunk c-1) with q-projection(chunk c) matmuls interleaved
  slot4  : attention(chunk 3) with o-projection(chunk 0) interleaved
  C      : o-projection chunks 1-3
Attention per (head, s-block): scores matmul -> exp on ACT engine (bf16 out) ->
ctx matmul (lagged one iteration so exp latency is hidden). The softmax
denominator never touches the tensor engine: a pairwise tree-add of exp tiles
on DVE, a GpSimd partition_all_reduce across s-partitions, DVE reciprocal, and
a DVE broadcast-multiply normalize ctx.

RoPE: wq/wk rows are permuted on the host so each head's (re, im) pairs sit 16
partitions apart within a 32-partition quadrant; stream_shuffle swaps them and
two multiplies + add with host-built cos/sin tables apply the rotation.
"""

import sys
import math

for _p in ("/opt/trn_rl_repo", "/root/.axon_site"):
    if _p not in sys.path:
        sys.path.insert(0, _p)

import numpy as np

T = 2048
D = 4096
N_HEADS = 32
N_KV = 8
HD = 128
N_CORES = 8
GQ = N_HEADS // 4   # q heads per core = 8
GKV = N_KV // 4     # kv heads per core = 2
N_REP = GQ // GKV   # 4
TCH = 512           # t-chunk
NCH = T // TCH      # 4
KT = D // 128       # 32 contraction tiles
NSB = T // 128      # 16 s-blocks
QD = GQ * HD        # 1024
KD = GKV * HD       # 256
SCALE = 1.0 / math.sqrt(HD)


def _build_program():
    import concourse.bass as bass
    import concourse.tile as tile
    from concourse import bacc, mybir
    from concourse.bass_isa import ReduceOp

    f32 = mybir.dt.float32
    bf16 = mybir.dt.bfloat16
    Exp = mybir.ActivationFunctionType.Exp

    nc = bacc.Bacc("TRN2", target_bir_lowering=False, debug=False,
                   num_devices=N_CORES)

    xT = nc.dram_tensor("xT", [D, T], bf16, kind="ExternalInput")
    wqT = nc.dram_tensor("wqT", [D, QD], bf16, kind="ExternalInput")
    wkvT = nc.dram_tensor("wkvT", [D, 2 * KD], bf16, kind="ExternalInput")
    woT = nc.dram_tensor("woT", [QD, D], bf16, kind="ExternalInput")
    C2 = nc.dram_tensor("C2", [128, T], bf16, kind="ExternalInput")
    S2m = nc.dram_tensor("S2m", [128, T], bf16, kind="ExternalInput")
    yT = nc.dram_tensor("yT", [D, T], f32, kind="ExternalOutput")

    SWAP = [(i + 16) % 32 for i in range(32)]  # swap 16-halves in each quadrant

    with tile.TileContext(nc) as tc, nc.allow_low_precision("bf16 kernel"):
        with tc.tile_pool(name="persist", bufs=1) as persist:
            k_sb = [persist.tile([128, T], bf16, name=f"k{m}", tag=f"k{m}")
                    for m in range(GKV)]
            v_sb = [persist.tile([128, KD], bf16, name=f"v{tb}", tag=f"v{tb}")
                    for tb in range(NSB)]
            ctx_sb = [[persist.tile([128, TCH], bf16, name=f"ctx{c}_{h}",
                                    tag=f"ctx{c}_{h}") for h in range(GQ)]
                      for c in range(NCH)]
            q_sb = [[persist.tile([128, TCH], bf16, name=f"q{p}_{h}",
                                  tag=f"q{p}_{h}") for h in range(GQ)]
                    for p in range(2)]

            def make_rope(ropep, c2_sb, s2m_sb):
                def rope_evac(ps, dst_ap, c):
                    c0, c1 = c * TCH, (c + 1) * TCH
                    t1 = ropep.tile([128, TCH], bf16, name="rt1", tag="rt1")
                    nc.vector.tensor_mul(t1[:], ps[:], c2_sb[:, c0:c1])
                    sh = ropep.tile([128, TCH], f32, name="rsh", tag="rsh")
                    nc.vector.stream_shuffle(sh[:], ps[:], SWAP)
                    t2 = ropep.tile([128, TCH], bf16, name="rt2", tag="rt2")
                    nc.vector.tensor_mul(t2[:], sh[:], s2m_sb[:, c0:c1])
                    nc.vector.tensor_add(dst_ap, t1[:], t2[:])
                return rope_evac

            def make_att(scp, ctxp, exps, rbp):
                """Emit one attention chunk with a filler callable interleaved.

                ctx matmuls lag TWO iterations behind their exp so the ACT
                chain latency never stalls the tensor engine (keeps the PE
                p-state ramped). Each head's 16 exp tiles land in one
                contiguous [128, NSB, TCH] buffer; the softmax denominator is
                a single DVE tensor_reduce over its transposed view, a GpSimd
                partition_all_reduce, and a fast approximate reciprocal.
                """
                state = {}
                pend = []

                def emit_ctx(ca, h, sb, ex):
                    st = state[h]
                    kv = h // N_REP
                    nc.tensor.matmul(st["ctx"][:], v_sb[sb][:, kv * 128:(kv + 1) * 128],
                                     ex, start=(sb == 0), stop=(sb == NSB - 1))
                    if sb == NSB - 1:
                        red = rbp.tile([128, TCH], f32, name="red", tag="red")
                        nc.vector.tensor_reduce(
                            red[:], st["exw"][:].transpose([0, 2, 1]),
                            mybir.AxisListType.X, mybir.AluOpType.add)
                        den_bc = rbp.tile([128, TCH], f32, name="denbc", tag="denbc")
                        nc.gpsimd.partition_all_reduce(den_bc[:], red[:],
                                                       128, ReduceOp.add)
                        rb = rbp.tile([128, TCH], f32, name="rb", tag="rb")
                        nc.vector.reciprocal_approx_fast(rb[:], den_bc[:])
                        nc.vector.tensor_mul(ctx_sb[ca][h][:], st["ctx"][:], rb[:])
                        del state[h]

                def att_chunk(ca, filler):
                    for i in range(GQ * NSB):
                        h, sb = divmod(i, NSB)
                        kv = h // N_REP
                        sc = scp.tile([128, TCH], f32, name="sc", tag="sc")
                        nc.tensor.matmul(sc[:], k_sb[kv][:, sb * 128:(sb + 1) * 128],
                                         q_sb[ca % 2][h][:], start=True, stop=True)
                        if sb == 0:
                            state[h] = {
                                "ctx": ctxp.tile([128, TCH], f32, name="ctxps", tag="ctxps"),
                                "exw": exps.tile([128, NSB, TCH], bf16,
                                                 name="exw", tag="exw"),
                            }
                        st = state[h]
                        ex = st["exw"][:, sb, :]
                        nc.scalar.activation(ex, sc[:], Exp, scale=SCALE)
                        filler(i)
                        while len(pend) >= 2:
                            emit_ctx(*pend.pop(0))
                        pend.append((ca, h, sb, ex))
                    # flush final pending ctx matmuls
                    while pend:
                        emit_ctx(*pend.pop(0))
                return att_chunk

            # ================= region 1: projections + attention ==========
            with tc.tile_pool(name="wsb", bufs=1) as wsb:
                wq_t = [wsb.tile([128, QD], bf16, name=f"wq{k}", tag=f"wq{k}")
                        for k in range(KT)]
                c2_sb = wsb.tile([128, T], bf16, tag="c2")
                nc.sync.dma_start(c2_sb[:], C2[:])
                s2m_sb = wsb.tile([128, T], bf16, tag="s2m")
                nc.sync.dma_start(s2m_sb[:], S2m[:])

                # ---------------- Stage A: k/v projections ---------------
                with tc.tile_pool(name="wkvp", bufs=1) as wkvp, \
                     tc.tile_pool(name="xa", bufs=8) as xap, \
                     tc.tile_pool(name="ropea", bufs=2) as ropea, \
                     tc.tile_pool(name="kvk", bufs=4, space="PSUM") as kvk, \
                     tc.tile_pool(name="kvv", bufs=4, space="PSUM") as kvv:
                    wkv_t = [wkvp.tile([128, 2 * KD], bf16, name=f"wkv{k}",
                                       tag=f"wkv{k}") for k in range(KT)]
                    rope_a = make_rope(ropea, c2_sb, s2m_sb)
                    for k in range(3):
                        nc.sync.dma_start(wkv_t[k][:],
                                          wkvT[k * 128:(k + 1) * 128, :])
                    for c in range(NCH):
                        c0, c1 = c * TCH, (c + 1) * TCH
                        kps = [kvk.tile([128, TCH], f32, name=f"kps{m}", tag="kps")
                               for m in range(GKV)]
                        vps = [kvv.tile([128, KD], f32, name=f"vps{tb}", tag="vps")
                               for tb in range(TCH // 128)]
                        for k in range(KT):
                            if c == 0 and k + 3 < KT:
                                # stream weight loads behind the first chunk's
                                # compute instead of blocking at program start
                                nc.sync.dma_start(wkv_t[k + 3][:],
                                                  wkvT[(k + 3) * 128:(k + 4) * 128, :])
                            elif c == 1:
                                nc.sync.dma_start(wq_t[k][:],
                                                  wqT[k * 128:(k + 1) * 128, :])
                            xt = xap.tile([128, TCH], bf16, name="xa", tag="xa")
                            nc.sync.dma_start(xt[:], xT[k * 128:(k + 1) * 128, c0:c1])
                            for m in range(GKV):
                                nc.tensor.matmul(kps[m][:],
                                                 wkv_t[k][:, m * 128:(m + 1) * 128],
                                                 xt[:], start=(k == 0), stop=(k == KT - 1))
                            for tb in range(TCH // 128):
                                nc.tensor.matmul(vps[tb][:],
                                                 xt[:, tb * 128:(tb + 1) * 128],
                                                 wkv_t[k][:, KD:], start=(k == 0),
                                                 stop=(k == KT - 1))
                        for m in range(GKV):
                            rope_a(kps[m], k_sb[m][:, c0:c1], c)
                        for tb in range(TCH // 128):
                            nc.scalar.copy(v_sb[c * (TCH // 128) + tb][:], vps[tb][:])

                # ------------- Stage B: q-proj + attention ----------------
                with tc.tile_pool(name="xb", bufs=7) as xbp, \
                     tc.tile_pool(name="ropeb", bufs=2) as ropeb, \
                     tc.tile_pool(name="exps", bufs=2) as exps, \
                     tc.tile_pool(name="rbp", bufs=2) as rbp, \
                     tc.tile_pool(name="qps", bufs=4, space="PSUM") as qpsp, \
                     tc.tile_pool(name="scp", bufs=2, space="PSUM") as scp, \
                     tc.tile_pool(name="ctxp", bufs=2, space="PSUM") as ctxp:
                    rope_b = make_rope(ropeb, c2_sb, s2m_sb)
                    att_chunk = make_att(scp, ctxp, exps, rbp)
                    qst = [None] * 4

                    def qproj_kstep(c, p, k):
                        c0, c1 = c * TCH, (c + 1) * TCH
                        if k == 0:
                            for j in range(4):
                                qst[j] = qpsp.tile([128, TCH], f32,
                                                   name=f"qps{j}", tag="qps")
                        xt = xbp.tile([128, TCH], bf16, name="xb", tag="xb")
                        nc.sync.dma_start(xt[:], xT[k * 128:(k + 1) * 128, c0:c1])
                        for j in range(4):
                            h = p * 4 + j
                            nc.tensor.matmul(qst[j][:],
                                             wq_t[k][:, h * 128:(h + 1) * 128],
                                             xt[:], start=(k == 0), stop=(k == KT - 1))
                        if k == KT - 1:
                            for j in range(4):
                                rope_b(qst[j], q_sb[c % 2][p * 4 + j][:], c)

                    # slot 0: q-projection chunk 0, nothing to interleave
                    for p in range(2):
                        for k in range(KT):
                            qproj_kstep(0, p, k)

                    # slots 1..3: attention(c-1) + q-projection(c)
                    for c in range(1, NCH):
                        def filler(i, c=c):
                            if i % 2 == 0:
                                qproj_kstep(c, i // 64, (i % 64) // 2)
                        att_chunk(c - 1, filler)

            # ============ region 2: last attention + o-projection =========
            with tc.tile_pool(name="wo", bufs=1) as wop, \
                 tc.tile_pool(name="exps2", bufs=2) as exps2, \
                 tc.tile_pool(name="rbp2", bufs=2) as rbp2, \
                 tc.tile_pool(name="out", bufs=4) as outp, \
                 tc.tile_pool(name="scp2", bufs=2, space="PSUM") as scp2, \
                 tc.tile_pool(name="ctxp2", bufs=2, space="PSUM") as ctxp2, \
                 tc.tile_pool(name="yps", bufs=4, space="PSUM") as yps:
                wo_t = [wop.tile([128, D], bf16, name=f"wo{hk}", tag=f"wo{hk}")
                        for hk in range(GQ)]
                for hk in range(GQ):
                    nc.sync.dma_start(wo_t[hk][:], woT[hk * 128:(hk + 1) * 128, :])

                ydone = [None]

                def oproj_mblock(m, c):
                    c0, c1 = c * TCH, (c + 1) * TCH
                    y_ps = yps.tile([128, TCH], f32, name="yps", tag="yps")
                    for hk in range(GQ):
                        nc.tensor.matmul(y_ps[:], wo_t[hk][:, m * 128:(m + 1) * 128],
                                         ctx_sb[c][hk][:], start=(hk == 0),
                                         stop=(hk == GQ - 1))
                    ot = outp.tile([128, TCH], f32, name="ot", tag="ot")
                    nc.scalar.copy(ot[:], y_ps[:])
                    nc.sync.dma_start(yT[m * 128:(m + 1) * 128, c0:c1], ot[:])

                ostate = [None]

                def oproj_part(i):
                    # two o-proj matmuls per attention iteration, chunk 0
                    m, sub = divmod(i, 4)
                    if sub == 0:
                        ostate[0] = yps.tile([128, TCH], f32, name="yps", tag="yps")
                    y_ps = ostate[0]
                    for hk in (2 * sub, 2 * sub + 1):
                        nc.tensor.matmul(y_ps[:], wo_t[hk][:, m * 128:(m + 1) * 128],
                                         ctx_sb[0][hk][:], start=(hk == 0),
                                         stop=(hk == GQ - 1))
                    if sub == 3:
                        ot = outp.tile([128, TCH], f32, name="ot", tag="ot")
                        nc.vector.tensor_copy(ot[:], y_ps[:])
                        nc.sync.dma_start(yT[m * 128:(m + 1) * 128, 0:TCH], ot[:])

                att_chunk2 = make_att(scp2, ctxp2, exps2, rbp2)
                att_chunk2(NCH - 1, oproj_part)

                # Stage C: o-projection chunks 1..3
                for c in range(1, NCH):
                    for m in range(KT):
                        oproj_mblock(m, c)

    nc.compile()
    return nc


_PROGRAM = None


def _get_program():
    global _PROGRAM
    if _PROGRAM is None:
        _PROGRAM = _build_program()
    return _PROGRAM


def _rope_perm():
    """Within-head row permutation: row 32*q + i  <-  component 2*(16q+i%16)+ (i>=16)."""
    perm = np.empty(HD, dtype=np.int64)
    for q in range(4):
        for i in range(32):
            j = 16 * q + (i % 16)
            perm[32 * q + i] = 2 * j + (1 if i >= 16 else 0)
    return perm


def _host_prep(x, wq, wk, wv, wo, cos, sin):
    """Build the per-core input maps."""
    import ml_dtypes
    bf16 = ml_dtypes.bfloat16
    perm = _rope_perm()
    f32 = np.float32

    cosT = np.ascontiguousarray(cos.T.astype(f32))   # [64, T]
    sinT = np.ascontiguousarray(sin.T.astype(f32))
    C2 = np.empty((128, T), f32)
    S2m = np.empty((128, T), f32)
    for q in range(4):
        for i in range(32):
            j = 16 * q + (i % 16)
            C2[32 * q + i] = cosT[j]
            S2m[32 * q + i] = sinT[j] if i >= 16 else -sinT[j]
    C2 = C2.astype(bf16)
    S2m = S2m.astype(bf16)

    in_maps = []
    for core in range(N_CORES):
        b, g = divmod(core, 4)
        qrows = np.concatenate([(8 * g + j) * HD + perm for j in range(GQ)])
        krows = np.concatenate([(2 * g + m) * HD + perm for m in range(GKV)])
        vrows = np.arange(2 * g * HD, (2 * g + 2) * HD)
        ocols = np.arange(8 * g * HD, (8 * g + 8) * HD)
        in_maps.append({
            "xT": np.ascontiguousarray(x[b].T).astype(bf16),
            "wqT": np.ascontiguousarray(wq[qrows].T).astype(bf16),
            "wkvT": np.ascontiguousarray(
                np.concatenate([wk[krows], wv[vrows]], axis=0).T).astype(bf16),
            "woT": np.ascontiguousarray(wo[:, ocols].T).astype(bf16),
            "C2": C2, "S2m": S2m,
        })
    return in_maps


def kernel(x, wq, wk, wv, wo, cache_k, cache_v, cos, sin, mask, start_pos):
    x = np.asarray(x)
    wq, wk, wv, wo = (np.asarray(a) for a in (wq, wk, wv, wo))
    cos, sin = np.asarray(cos), np.asarray(sin)
    assert int(start_pos) == 0, "kernel hardcodes start_pos == 0"
    assert x.shape == (2, T, D)

    from concourse.bass_utils import run_bass_kernel_spmd

    nc = _get_program()
    in_maps = _host_prep(x, wq, wk, wv, wo, cos, sin)
    res = run_bass_kernel_spmd(nc, in_maps, list(range(N_CORES)))

    y = np.empty((2, T, D), np.float32)
    for b in range(2):
        acc = res.results[4 * b]["yT"].copy()
        for g in range(1, 4):
            acc += res.results[4 * b + g]["yT"]
        y[b] = acc.T
    return y


# revision 14
# speedup vs baseline: 1.6468x; 1.2625x over previous
"""Self-contained Trainium2 Bass kernel for GQA attention (B=2, T=2048, D=4096,
32 q heads / 8 kv heads, HD=128, RoPE, no causal mask, start_pos=0).

Sharding: 8 cores = 2 (batch) x 4 (head groups). Each core computes 8 q heads /
2 kv heads for one batch and a partial o-projection; the host sums the 4
partials per batch.

All matmul operands are bf16 (PSUM accumulation stays f32), which halves DMA
traffic and lets every weight stay SBUF-resident (loaded exactly once).

Schedule (single core), tuned so the tensor engine never waits on the
exp-activation engine:
  A      : k/v projections + RoPE for all 4 t-chunks (x streamed, w resident)
  slot0  : q projection chunk 0
  slot1-3: attention(chunk c-1) with q-projection(chunk c) matmuls interleaved
  slot4  : attention(chunk 3) with o-projection(chunk 0) interleaved
  C      : o-projection chunks 1-3
Attention per (head, s-block): scores matmul -> exp on ACT engine (bf16 out) ->
ctx matmul (lagged one iteration so exp latency is hidden). The softmax
denominator never touches the tensor engine: a pairwise tree-add of exp tiles
on DVE, a GpSimd partition_all_reduce across s-partitions, DVE reciprocal, and
a DVE broadcast-multiply normalize ctx.

RoPE: wq/wk rows are permuted on the host so each head's (re, im) pairs sit 16
partitions apart within a 32-partition quadrant; stream_shuffle swaps them and
two multiplies + add with host-built cos/sin tables apply the rotation.
"""

import sys
import math

for _p in ("/opt/trn_rl_repo", "/root/.axon_site"):
    if _p not in sys.path:
        sys.path.insert(0, _p)

import numpy as np

T = 2048
D = 4096
N_HEADS = 32
N_KV = 8
HD = 128
N_CORES = 8
GQ = N_HEADS // 4   # q heads per core = 8
GKV = N_KV // 4     # kv heads per core = 2
N_REP = GQ // GKV   # 4
TCH = 512           # t-chunk
NCH = T // TCH      # 4
KT = D // 128       # 32 contraction tiles
NSB = T // 128      # 16 s-blocks
QD = GQ * HD        # 1024
KD = GKV * HD       # 256
SCALE = 1.0 / math.sqrt(HD)


def _build_program():
    import concourse.bass as bass
    import concourse.tile as tile
    from concourse import bacc, mybir
    from concourse.bass_isa import ReduceOp

    f32 = mybir.dt.float32
    bf16 = mybir.dt.bfloat16
    Exp = mybir.ActivationFunctionType.Exp

    nc = bacc.Bacc("TRN2", target_bir_lowering=False, debug=False,
                   num_devices=N_CORES)

    xT = nc.dram_tensor("xT", [D, T], bf16, kind="ExternalInput")
    wqT = nc.dram_tensor("wqT", [D, QD], bf16, kind="ExternalInput")
    wkvT = nc.dram_tensor("wkvT", [D, 2 * KD], bf16, kind="ExternalInput")
    woT = nc.dram_tensor("woT", [QD, D], bf16, kind="ExternalInput")
    C2 = nc.dram_tensor("C2", [128, T], bf16, kind="ExternalInput")
    S2m = nc.dram_tensor("S2m", [128, T], bf16, kind="ExternalInput")
    yT = nc.dram_tensor("yT", [D, T], f32, kind="ExternalOutput")

    SWAP = [(i + 16) % 32 for i in range(32)]  # swap 16-halves in each quadrant

    with tile.TileContext(nc) as tc, nc.allow_low_precision("bf16 kernel"):
        with tc.tile_pool(name="persist", bufs=1) as persist:
            k_sb = [persist.tile([128, T], bf16, name=f"k{m}", tag=f"k{m}")
                    for m in range(GKV)]
            v_sb = [persist.tile([128, KD], bf16, name=f"v{tb}", tag=f"v{tb}")
                    for tb in range(NSB)]
            ctx_sb = [[persist.tile([128, TCH], bf16, name=f"ctx{c}_{h}",
                                    tag=f"ctx{c}_{h}") for h in range(GQ)]
                      for c in range(NCH)]
            q_sb = [[persist.tile([128, TCH], bf16, name=f"q{p}_{h}",
                                  tag=f"q{p}_{h}") for h in range(GQ)]
                    for p in range(2)]

            def make_rope(ropep, c2_sb, s2m_sb):
                def rope_evac(ps, dst_ap, c):
                    c0, c1 = c * TCH, (c + 1) * TCH
                    t1 = ropep.tile([128, TCH], bf16, name="rt1", tag="rt1")
                    nc.vector.tensor_mul(t1[:], ps[:], c2_sb[:, c0:c1])
                    sh = ropep.tile([128, TCH], f32, name="rsh", tag="rsh")
                    nc.vector.stream_shuffle(sh[:], ps[:], SWAP)
                    t2 = ropep.tile([128, TCH], bf16, name="rt2", tag="rt2")
                    nc.vector.tensor_mul(t2[:], sh[:], s2m_sb[:, c0:c1])
                    nc.vector.tensor_add(dst_ap, t1[:], t2[:])
                return rope_evac

            def make_att(scp, ctxp, exps, rbp):
                """Emit one attention chunk with a filler callable interleaved.

                ctx matmuls lag TWO iterations behind their exp so the ACT
                chain latency never stalls the tensor engine (keeps the PE
                p-state ramped). Each head's 16 exp tiles land in one
                contiguous [128, NSB, TCH] buffer; the softmax denominator is
                a single DVE tensor_reduce over its transposed view, a GpSimd
                partition_all_reduce, and a fast approximate reciprocal.
                """
                state = {}
                pend = []

                def emit_ctx(ca, h, sb, ex):
                    st = state[h]
                    kv = h // N_REP
                    nc.tensor.matmul(st["ctx"][:], v_sb[sb][:, kv * 128:(kv + 1) * 128],
                                     ex, start=(sb == 0), stop=(sb == NSB - 1))
                    if sb == NSB - 1:
                        # wide in-place halving adds: one data pass on DVE in
                        # 4 instructions (every ctx matmul is already emitted)
                        exw = st["exw"]
                        nc.vector.tensor_add(exw[:, 0:8, :], exw[:, 0:8, :],
                                             exw[:, 8:16, :])
                        nc.vector.tensor_add(exw[:, 0:4, :], exw[:, 0:4, :],
                                             exw[:, 4:8, :])
                        nc.vector.tensor_add(exw[:, 0:2, :], exw[:, 0:2, :],
                                             exw[:, 2:4, :])
                        red = rbp.tile([128, TCH], f32, name="red", tag="red")
                        nc.vector.tensor_add(red[:], exw[:, 0, :], exw[:, 1, :])
                        den_bc = rbp.tile([128, TCH], f32, name="denbc", tag="denbc")
                        nc.gpsimd.partition_all_reduce(den_bc[:], red[:],
                                                       128, ReduceOp.add)
                        rb = rbp.tile([128, TCH], f32, name="rb", tag="rb")
                        nc.vector.reciprocal_approx_fast(rb[:], den_bc[:])
                        nc.vector.tensor_mul(ctx_sb[ca][h][:], st["ctx"][:], rb[:])
                        del state[h]

                def att_chunk(ca, filler):
                    for i in range(GQ * NSB):
                        h, sb = divmod(i, NSB)
                        kv = h // N_REP
                        sc = scp.tile([128, TCH], f32, name="sc", tag="sc")
                        nc.tensor.matmul(sc[:], k_sb[kv][:, sb * 128:(sb + 1) * 128],
                                         q_sb[ca % 2][h][:], start=True, stop=True)
                        if sb == 0:
                            state[h] = {
                                "ctx": ctxp.tile([128, TCH], f32, name="ctxps", tag="ctxps"),
                                "exw": exps.tile([128, NSB, TCH], bf16,
                                                 name="exw", tag="exw"),
                            }
                        st = state[h]
                        ex = st["exw"][:, sb, :]
                        nc.scalar.activation(ex, sc[:], Exp, scale=SCALE)
                        filler(i)
                        while len(pend) >= 2:
                            emit_ctx(*pend.pop(0))
                        pend.append((ca, h, sb, ex))
                    # flush final pending ctx matmuls
                    while pend:
                        emit_ctx(*pend.pop(0))
                return att_chunk

            # ======== single region: projections + attention + o-proj =====
            with tc.tile_pool(name="cst", bufs=1) as cstp, \
                 tc.tile_pool(name="rope", bufs=2) as ropep:
                c2_sb = cstp.tile([128, T], bf16, tag="c2")
                nc.sync.dma_start(c2_sb[:], C2[:])
                s2m_sb = cstp.tile([128, T], bf16, tag="s2m")
                nc.sync.dma_start(s2m_sb[:], S2m[:])
                rope = make_rope(ropep, c2_sb, s2m_sb)

                # ---------------- Stage A: k/v projections ---------------
                with tc.tile_pool(name="wkvp", bufs=1) as wkvp, \
                     tc.tile_pool(name="xa", bufs=10) as xap, \
                     tc.tile_pool(name="kvk", bufs=4, space="PSUM") as kvk, \
                     tc.tile_pool(name="kvv", bufs=4, space="PSUM") as kvv:
                    wkv_t = [wkvp.tile([128, 2 * KD], bf16, name=f"wkv{k}",
                                       tag=f"wkv{k}") for k in range(KT)]
                    for k in range(3):
                        nc.sync.dma_start(wkv_t[k][:],
                                          wkvT[k * 128:(k + 1) * 128, :])
                    for c in range(NCH):
                        c0, c1 = c * TCH, (c + 1) * TCH
                        kps = [kvk.tile([128, TCH], f32, name=f"kps{m}", tag="kps")
                               for m in range(GKV)]
                        vps = [kvv.tile([128, KD], f32, name=f"vps{tb}", tag="vps")
                               for tb in range(TCH // 128)]
                        for k in range(KT):
                            if c == 0 and k + 3 < KT:
                                # stream weight loads behind the first chunk's
                                # compute instead of blocking at program start
                                nc.sync.dma_start(wkv_t[k + 3][:],
                                                  wkvT[(k + 3) * 128:(k + 4) * 128, :])
                            xt = xap.tile([128, TCH], bf16, name="xa", tag="xa")
                            nc.sync.dma_start(xt[:], xT[k * 128:(k + 1) * 128, c0:c1])
                            for m in range(GKV):
                                nc.tensor.matmul(kps[m][:],
                                                 wkv_t[k][:, m * 128:(m + 1) * 128],
                                                 xt[:], start=(k == 0), stop=(k == KT - 1))
                            for tb in range(TCH // 128):
                                nc.tensor.matmul(vps[tb][:],
                                                 xt[:, tb * 128:(tb + 1) * 128],
                                                 wkv_t[k][:, KD:], start=(k == 0),
                                                 stop=(k == KT - 1))
                        for m in range(GKV):
                            rope(kps[m], k_sb[m][:, c0:c1], c)
                        for tb in range(TCH // 128):
                            nc.scalar.copy(v_sb[c * (TCH // 128) + tb][:], vps[tb][:])

                # ------- Stage B/C pools (span attention + o-proj) --------
                with tc.tile_pool(name="exps", bufs=2) as exps, \
                     tc.tile_pool(name="rbp", bufs=2) as rbp, \
                     tc.tile_pool(name="scp", bufs=2, space="PSUM") as scp, \
                     tc.tile_pool(name="ctxp", bufs=2, space="PSUM") as ctxp:
                    att_chunk = make_att(scp, ctxp, exps, rbp)

                    # ---- slots 0..3: q-projection + attention ------------
                    with tc.tile_pool(name="wqp", bufs=1) as wqp, \
                         tc.tile_pool(name="xb", bufs=8) as xbp, \
                         tc.tile_pool(name="qps", bufs=4, space="PSUM") as qpsp:
                        wq_t = [wqp.tile([128, QD], bf16, name=f"wq{k}",
                                         tag=f"wq{k}") for k in range(KT)]
                        for k in range(3):
                            nc.sync.dma_start(wq_t[k][:],
                                              wqT[k * 128:(k + 1) * 128, :])
                        qst = [None] * 4

                        def qproj_kstep(c, p, k):
                            c0, c1 = c * TCH, (c + 1) * TCH
                            if c == 0 and p == 0 and k + 3 < KT:
                                nc.sync.dma_start(wq_t[k + 3][:],
                                                  wqT[(k + 3) * 128:(k + 4) * 128, :])
                            if k == 0:
                                for j in range(4):
                                    qst[j] = qpsp.tile([128, TCH], f32,
                                                       name=f"qps{j}", tag="qps")
                            xt = xbp.tile([128, TCH], bf16, name="xb", tag="xb")
                            nc.sync.dma_start(xt[:], xT[k * 128:(k + 1) * 128, c0:c1])
                            for j in range(4):
                                h = p * 4 + j
                                nc.tensor.matmul(qst[j][:],
                                                 wq_t[k][:, h * 128:(h + 1) * 128],
                                                 xt[:], start=(k == 0), stop=(k == KT - 1))
                            if k == KT - 1:
                                for j in range(4):
                                    rope(qst[j], q_sb[c % 2][p * 4 + j][:], c)

                        # slot 0: q-projection chunk 0, nothing to interleave
                        for p in range(2):
                            for k in range(KT):
                                qproj_kstep(0, p, k)

                        # slots 1..3: attention(c-1) + q-projection(c)
                        for c in range(1, NCH):
                            def filler(i, c=c):
                                if i % 2 == 0:
                                    qproj_kstep(c, i // 64, (i % 64) // 2)
                            att_chunk(c - 1, filler)

                    # ---- slot 4 + stage C: o-projection ------------------
                    with tc.tile_pool(name="wo", bufs=1) as wop, \
                         tc.tile_pool(name="out", bufs=4) as outp, \
                         tc.tile_pool(name="yps", bufs=4, space="PSUM") as yps:
                        # wo in two half-panels (m 0..15 / m 16..31), both
                        # prefetched behind attention(3)
                        wo_t = [[wop.tile([128, KT * 64], bf16, name=f"wo{p}_{hk}",
                                          tag=f"wo{p}_{hk}") for hk in range(GQ)]
                                for p in range(2)]
                        for p in range(2):
                            for hk in range(GQ):
                                nc.sync.dma_start(
                                    wo_t[p][hk][:],
                                    woT[hk * 128:(hk + 1) * 128,
                                        p * (D // 2):(p + 1) * (D // 2)])

                        def wo_ap(m, hk):
                            p, ml = divmod(m, KT // 2)
                            return wo_t[p][hk][:, ml * 128:(ml + 1) * 128]

                        def oproj_mblock(m, c):
                            c0, c1 = c * TCH, (c + 1) * TCH
                            y_ps = yps.tile([128, TCH], f32, name="yps", tag="yps")
                            for hk in range(GQ):
                                nc.tensor.matmul(y_ps[:], wo_ap(m, hk),
                                                 ctx_sb[c][hk][:], start=(hk == 0),
                                                 stop=(hk == GQ - 1))
                            ot = outp.tile([128, TCH], f32, name="ot", tag="ot")
                            nc.scalar.copy(ot[:], y_ps[:])
                            nc.sync.dma_start(yT[m * 128:(m + 1) * 128, c0:c1], ot[:])

                        ostate = [None]

                        def oproj_part(i):
                            # two o-proj matmuls per attention iteration:
                            # m 0..15 of chunks 0 and 1
                            c, j = divmod(i, 64)
                            m, sub = divmod(j, 4)
                            if sub == 0:
                                ostate[0] = yps.tile([128, TCH], f32,
                                                     name="yps", tag="yps")
                            y_ps = ostate[0]
                            for hk in (2 * sub, 2 * sub + 1):
                                nc.tensor.matmul(y_ps[:], wo_ap(m, hk),
                                                 ctx_sb[c][hk][:], start=(hk == 0),
                                                 stop=(hk == GQ - 1))
                            if sub == 3:
                                ot = outp.tile([128, TCH], f32, name="ot", tag="ot")
                                nc.vector.tensor_copy(ot[:], y_ps[:])
                                nc.sync.dma_start(
                                    yT[m * 128:(m + 1) * 128,
                                       c * TCH:(c + 1) * TCH], ot[:])

                        att_chunk(NCH - 1, oproj_part)

                        # stage C: remaining o-projection blocks
                        for c in (2, 3):
                            for m in range(KT // 2):
                                oproj_mblock(m, c)
                        for c in range(NCH):
                            for m in range(KT // 2, KT):
                                oproj_mblock(m, c)

    nc.compile()
    return nc


_PROGRAM = None


def _get_program():
    global _PROGRAM
    if _PROGRAM is None:
        _PROGRAM = _build_program()
    return _PROGRAM


def _rope_perm():
    """Within-head row permutation: row 32*q + i  <-  component 2*(16q+i%16)+ (i>=16)."""
    perm = np.empty(HD, dtype=np.int64)
    for q in range(4):
        for i in range(32):
            j = 16 * q + (i % 16)
            perm[32 * q + i] = 2 * j + (1 if i >= 16 else 0)
    return perm


def _host_prep(x, wq, wk, wv, wo, cos, sin):
    """Build the per-core input maps."""
    import ml_dtypes
    bf16 = ml_dtypes.bfloat16
    perm = _rope_perm()
    f32 = np.float32

    cosT = np.ascontiguousarray(cos.T.astype(f32))   # [64, T]
    sinT = np.ascontiguousarray(sin.T.astype(f32))
    C2 = np.empty((128, T), f32)
    S2m = np.empty((128, T), f32)
    for q in range(4):
        for i in range(32):
            j = 16 * q + (i % 16)
            C2[32 * q + i] = cosT[j]
            S2m[32 * q + i] = sinT[j] if i >= 16 else -sinT[j]
    C2 = C2.astype(bf16)
    S2m = S2m.astype(bf16)

    in_maps = []
    for core in range(N_CORES):
        b, g = divmod(core, 4)
        qrows = np.concatenate([(8 * g + j) * HD + perm for j in range(GQ)])
        krows = np.concatenate([(2 * g + m) * HD + perm for m in range(GKV)])
        vrows = np.arange(2 * g * HD, (2 * g + 2) * HD)
        ocols = np.arange(8 * g * HD, (8 * g + 8) * HD)
        in_maps.append({
            "xT": np.ascontiguousarray(x[b].T).astype(bf16),
            "wqT": np.ascontiguousarray(wq[qrows].T).astype(bf16),
            "wkvT": np.ascontiguousarray(
                np.concatenate([wk[krows], wv[vrows]], axis=0).T).astype(bf16),
            "woT": np.ascontiguousarray(wo[:, ocols].T).astype(bf16),
            "C2": C2, "S2m": S2m,
        })
    return in_maps


def kernel(x, wq, wk, wv, wo, cache_k, cache_v, cos, sin, mask, start_pos):
    x = np.asarray(x)
    wq, wk, wv, wo = (np.asarray(a) for a in (wq, wk, wv, wo))
    cos, sin = np.asarray(cos), np.asarray(sin)
    assert int(start_pos) == 0, "kernel hardcodes start_pos == 0"
    assert x.shape == (2, T, D)

    from concourse.bass_utils import run_bass_kernel_spmd

    nc = _get_program()
    in_maps = _host_prep(x, wq, wk, wv, wo, cos, sin)
    res = run_bass_kernel_spmd(nc, in_maps, list(range(N_CORES)))

    y = np.empty((2, T, D), np.float32)
    for b in range(2):
        acc = res.results[4 * b]["yT"].copy()
        for g in range(1, 4):
            acc += res.results[4 * b + g]["yT"]
        y[b] = acc.T
    return y
